# revision 13
# baseline (speedup 1.0000x reference)
"""DSAttention Trainium2 kernel (8 NeuronCores, SPMD).

Sharding: batch (B=2) x head-groups (4 heads each) -> 8 cores.
Core c handles batch b=c//4, heads 4*(c%4) .. 4*(c%4)+3.

Per-core math (feature-major "transposed" layouts so softmax bias/scale land
on partition axes):
  q_t = Wq_p @ hs_b.T          [256, 2048]   (+bq per-partition)
  k_t = Wk_p @ hs_b.T          [256, 2048]   (+bk per-partition)
  v   = hs_b @ Wv_p.T          [2048, 256]   (per k-tile, with a ones column
                                              per head -> softmax denominator)
  s_t[k, q] = k_t.T q_t        per head, one k-tile x all 2048 q at a time
  e_t = exp(s_t * tau/8 + delta_k/8)         (fused ACT exp, N=1024 halves;
                                              no max-subtraction: |logits|<~12)
  ctx_t[65, q] = [v | 1].T @ e_t             accumulated over 16 k-tiles;
                                              row 64 = denominator
  ctx_t[0:64] *= 1/ctx_t[64]                 (PE rank-1 broadcast of d, then
                                              64-lane DVE reciprocal + mul)
  out_partial = ctx.T @ Wo_p.T               [2048, 1024]
Host: out[b] = sum of the 4 head-group partials + bv @ Wo.T + bo
(softmax rows sum to 1, so the v/out biases commute to the host exactly).

All matmuls in float32r (~1.2e-4 input rounding, full PE rate at N>=256).
Phase B is software-pipelined: ctx matmuls for k-tile kt-1 are emitted after
the scores matmuls for kt so the PE queue never drains waiting on ACT.
"""

import sys

for _p in ("/opt/trn_rl_repo", "/opt/pypackages"):
    if _p not in sys.path:
        sys.path.append(_p)

import numpy as np

import concourse.bass as bass
import concourse.tile as tile
from concourse import bacc, mybir
from concourse.bass_utils import run_bass_kernel_spmd

B, L, H = 2, 2048, 1024
NH, HD = 16, 64
NCORES = 8
HPC = 4  # heads per core
FPC = HPC * HD  # 256
NKT = L // 128  # 16 k-tiles
NHC = H // 128  # 8 H-contraction chunks

F32 = mybir.dt.float32
F32R = mybir.dt.float32r

_NC_CACHE = {}


def _build_kernel():
    nc = bacc.Bacc(None, target_bir_lowering=False, debug=False)

    hs_t = nc.declare_dram_parameter("hs_t", [H, L], F32, isOutput=False)
    wq_t = nc.declare_dram_parameter("wq_t", [H, FPC], F32, isOutput=False)
    wk_t = nc.declare_dram_parameter("wk_t", [H, FPC], F32, isOutput=False)
    wv_t = nc.declare_dram_parameter("wv_t", [H, FPC], F32, isOutput=False)
    wo_t = nc.declare_dram_parameter("wo_t", [FPC, H], F32, isOutput=False)
    bq2 = nc.declare_dram_parameter("bq2", [128, 2], F32, isOutput=False)
    bk2 = nc.declare_dram_parameter("bk2", [128, 2], F32, isOutput=False)
    tau8 = nc.declare_dram_parameter("tau8", [128, 1], F32, isOutput=False)
    delta8 = nc.declare_dram_parameter("delta8", [128, NKT], F32, isOutput=False)
    out = nc.declare_dram_parameter("out", [L, H], F32, isOutput=True)

    with tile.TileContext(nc) as tc:
        with (
            tc.tile_pool(name="persist", bufs=1) as persist,
            tc.tile_pool(name="hsw", bufs=1) as hsw,
            # PSUM: "sc" 2 x [128,1024] slots (4 banks) + "ctx" 4 x 2KB (4 banks)
            tc.tile_pool(name="sc_ps", bufs=2, space="PSUM") as sc_ps,
            tc.tile_pool(name="ctx_ps", bufs=4, space="PSUM") as ctx_ps,
            tc.tile_pool(name="work", bufs=4) as work,
            tc.tile_pool(name="dscratch", bufs=2, space="DRAM") as dscratch,
        ):
            # ---- input loads -------------------------------------------------
            hs_sb = []
            for c in range(NHC):
                t = hsw.tile([128, L], F32R, tag=f"hs{c}", name=f"hs{c}")
                nc.sync.dma_start(out=t[:], in_=hs_t[c * 128 : (c + 1) * 128, :].bitcast(F32R))
                hs_sb.append(t)
            w_sb = {}
            for name, w in (("q", wq_t), ("k", wk_t), ("v", wv_t)):
                tiles = []
                for c in range(NHC):
                    t = hsw.tile([128, FPC], F32R, tag=f"w{name}{c}", name=f"w{name}{c}")
                    nc.gpsimd.dma_start(out=t[:], in_=w[c * 128 : (c + 1) * 128, :].bitcast(F32R))
                    tiles.append(t)
                w_sb[name] = tiles
            wo_sb = []
            for c in range(2):
                t = persist.tile([128, H], F32R, tag=f"wo{c}", name=f"wo{c}")
                nc.gpsimd.dma_start(out=t[:], in_=wo_t[c * 128 : (c + 1) * 128, :].bitcast(F32R))
                wo_sb.append(t)
            bq_sb = persist.tile([128, 2], F32, tag="bq")
            nc.sync.dma_start(out=bq_sb[:], in_=bq2[:])
            bk_sb = persist.tile([128, 2], F32, tag="bk")
            nc.sync.dma_start(out=bk_sb[:], in_=bk2[:])
            tau_sb = persist.tile([128, 1], F32, tag="tau")
            nc.sync.dma_start(out=tau_sb[:], in_=tau8[:])
            del8_sb = persist.tile([128, NKT], F32, tag="del8")
            nc.sync.dma_start(out=del8_sb[:], in_=delta8[:])
            vones_f = persist.tile([128, HPC], F32, tag="vones_f")
            nc.vector.memset(vones_f[:], 1.0)

            # ---- phase A: projections ---------------------------------------
            q_sb = [persist.tile([128, L], F32R, tag=f"q{hp}", name=f"q{hp}") for hp in range(2)]
            k_sb = [persist.tile([128, L], F32R, tag=f"k{hp}", name=f"k{hp}") for hp in range(2)]
            for dst, wname, bias in ((q_sb, "q", bq_sb), (k_sb, "k", bk_sb)):
                for hp in range(2):
                    for half in range(2):
                        off = half * 1024
                        ps = sc_ps.tile([128, 1024], F32, tag="sc", name="ps_proj")
                        for c in range(NHC):
                            for s2 in range(2):
                                nc.tensor.matmul(
                                    ps[:, s2 * 512 : (s2 + 1) * 512],
                                    w_sb[wname][c][:, hp * 128 : (hp + 1) * 128],
                                    hs_sb[c][:, off + s2 * 512 : off + (s2 + 1) * 512],
                                    start=(c == 0),
                                    stop=(c == NHC - 1),
                                )
                        nc.vector.tensor_scalar_add(
                            dst[hp][:, off : off + 1024],
                            ps[:],
                            bias[:, hp : hp + 1],
                        )

            # v: per k-tile [128, 4*65]; head h cols h*65..h*65+63, col h*65+64 = 1
            v_sb = [persist.tile([128, HPC * 65], F32R, tag=f"v{kt}", name=f"v{kt}") for kt in range(NKT)]
            for kt in range(NKT):
                ps = ctx_ps.tile([128, FPC], F32, tag="ctx", name="ps_vproj")
                for c in range(NHC):
                    nc.tensor.matmul(
                        ps[:],
                        hs_sb[c][:, kt * 128 : (kt + 1) * 128],
                        w_sb["v"][c][:],
                        start=(c == 0),
                        stop=(c == NHC - 1),
                    )
                v_view = v_sb[kt][:].rearrange("p (h w) -> p h w", h=HPC)
                nc.vector.tensor_copy(
                    v_view[:, :, 0:HD],
                    ps[:].rearrange("p (h w) -> p h w", h=HPC),
                )
                nc.vector.tensor_copy(v_view[:, :, HD : HD + 1].squeeze(), vones_f[:])

            # ---- phase B + C interleaved ------------------------------------
            # Outer loop over q-column pair-groups (gp: cols gp*1024 ..
            # gp*1024+1024). After a gp finishes all 4 heads, its 8 L-tiles of
            # the output projection are complete in ctx_sb; that C chunk is
            # emitted a few k-tiles INTO the next gp's first head so the
            # in-order PE queue never stalls on the normalize tail.
            ctx_sb = [persist.tile([128, L], F32R, tag=f"ctx{hp}", name=f"ctx{hp}") for hp in range(2)]
            pending_c = []

            def emit_c_chunk(lts):
                for lt in lts:
                    pso = [
                        ctx_ps.tile([128, 512], F32, tag="ctx", name=f"ps_o{nch}")
                        for nch in range(2)
                    ]
                    for c in range(2):
                        for nch in range(2):
                            nc.tensor.matmul(
                                pso[nch][:],
                                ctx_sb[c][:, lt * 128 : (lt + 1) * 128],
                                wo_sb[c][:, nch * 512 : (nch + 1) * 512],
                                start=(c == 0),
                                stop=(c == 1),
                            )
                    for nch in range(2):
                        o_sb = work.tile([128, 512], F32, tag="ostage", name="o_sb", bufs=3)
                        nc.vector.tensor_copy(o_sb[:], pso[nch][:])
                        nc.sync.dma_start(
                            out=out[lt * 128 : (lt + 1) * 128, nch * 512 : (nch + 1) * 512],
                            in_=o_sb[:],
                        )

            for gp in range(2):
                qoff = gp * 1024
                for h in range(HPC):
                    hp, hr = divmod(h, 2)
                    q_head = q_sb[hp][hr * HD : (hr + 1) * HD, :]
                    k_head = k_sb[hp][hr * HD : (hr + 1) * HD, :]
                    ctx2 = [
                        ctx_ps.tile([65, 512], F32, tag="ctx", name=f"ctx_g{gp}h{h}{g2}")
                        for g2 in range(2)
                    ]
                    prev = None  # (kt, e)

                    def emit_ctx(prev, h=h, ctx2=ctx2):
                        kt0, e = prev
                        for g2 in range(2):
                            nc.tensor.matmul(
                                ctx2[g2][:],
                                v_sb[kt0][:, h * 65 : (h + 1) * 65],
                                e[:, g2 * 512 : (g2 + 1) * 512],
                                start=(kt0 == 0),
                                stop=(kt0 == NKT - 1),
                            )

                    for kt in range(NKT):
                        psS = sc_ps.tile([128, 1024], F32, tag="sc", name="ps_s")
                        for s2 in range(2):
                            nc.tensor.matmul(
                                psS[:, s2 * 512 : (s2 + 1) * 512],
                                k_head[:, kt * 128 : (kt + 1) * 128],
                                q_head[:, qoff + s2 * 512 : qoff + (s2 + 1) * 512],
                                start=True,
                                stop=True,
                            )
                        if h == 0 and kt == 4 and pending_c:
                            emit_c_chunk(pending_c)
                            pending_c = []
                        if prev is not None:
                            emit_ctx(prev)
                        e_t = work.tile([128, 1024], F32R, tag="e", name="e_t", bufs=3)
                        nc.scalar.activation(
                            e_t[:],
                            psS[:],
                            mybir.ActivationFunctionType.Exp,
                            bias=del8_sb[:, kt : kt + 1],
                            scale=tau_sb[:],
                        )
                        prev = (kt, e_t)
                    emit_ctx(prev)

                    # normalize ctx[0:64] / ctx[64]: drain PSUM -> SBUF at once
                    # (frees accumulator banks so the next head's in-order PE
                    # queue never stalls), then broadcast the denominator row
                    # via a DRAM-bounce DMA and divide on DVE — no PE/PSUM.
                    raws = []
                    for g2 in range(2):
                        raw = work.tile([65, 512], F32R, tag="raw", name=f"raw{g2}")
                        nc.vector.tensor_copy(raw[:], ctx2[g2][:])
                        raws.append(raw)
                    for g2 in range(2):
                        d_dram = dscratch.tile([1, 512], F32, tag="ddram", name="d_dram")
                        nc.sync.dma_start(out=d_dram[:], in_=raws[g2][64:65, :].bitcast(F32))
                        d_bc = work.tile([64, 512], F32, tag="dbc", name="d_bc", bufs=2)
                        nc.sync.dma_start(
                            out=d_bc[:],
                            in_=d_dram[0:1, :].to_broadcast([64, 512]),
                        )
                        r_sb = work.tile([64, 512], F32, tag="r", name="r_sb", bufs=2)
                        nc.vector.reciprocal(r_sb[:], d_bc[:])
                        nc.vector.tensor_mul(
                            ctx_sb[hp][hr * HD : (hr + 1) * HD, qoff + g2 * 512 : qoff + (g2 + 1) * 512],
                            raws[g2][0:64, :],
                            r_sb[:],
                        )
                pending_c = list(range(gp * 8, (gp + 1) * 8))
            emit_c_chunk(pending_c)

    nc.compile()
    return nc


def _get_nc():
    if "nc" not in _NC_CACHE:
        _NC_CACHE["nc"] = _build_kernel()
    return _NC_CACHE["nc"]


def _make_in_maps(hidden_states, tau, delta, Wq, Wk, Wv, Wo, bq, bk):
    in_maps = []
    for c in range(NCORES):
        b, hg = divmod(c, HPC)
        fs = slice(hg * FPC, (hg + 1) * FPC)
        in_maps.append(
            {
                "hs_t": np.ascontiguousarray(hidden_states[b].T),
                "wq_t": np.ascontiguousarray(Wq[fs, :].T),
                "wk_t": np.ascontiguousarray(Wk[fs, :].T),
                "wv_t": np.ascontiguousarray(Wv[fs, :].T),
                "wo_t": np.ascontiguousarray(Wo[:, fs].T),
                "bq2": np.ascontiguousarray(bq[fs].reshape(2, 128).T),
                "bk2": np.ascontiguousarray(bk[fs].reshape(2, 128).T),
                "tau8": np.full((128, 1), tau[b, 0] / 8.0, dtype=np.float32),
                "delta8": np.ascontiguousarray((delta[b] / 8.0).reshape(NKT, 128).T),
            }
        )
    return in_maps


def kernel(hidden_states, tau, delta, Wq, bq, Wk, bk, Wv, bv, Wo, bo, _trace=False):
    hidden_states = np.asarray(hidden_states, dtype=np.float32)
    tau = np.asarray(tau, dtype=np.float32)
    delta = np.asarray(delta, dtype=np.float32)
    Wq = np.asarray(Wq, dtype=np.float32)
    Wk = np.asarray(Wk, dtype=np.float32)
    Wv = np.asarray(Wv, dtype=np.float32)
    Wo = np.asarray(Wo, dtype=np.float32)
    bq = np.asarray(bq, dtype=np.float32)
    bk = np.asarray(bk, dtype=np.float32)
    bv = np.asarray(bv, dtype=np.float32)
    bo = np.asarray(bo, dtype=np.float32)

    nc = _get_nc()
    in_maps = _make_in_maps(hidden_states, tau, delta, Wq, Wk, Wv, Wo, bq, bk)
    res = run_bass_kernel_spmd(nc, in_maps, list(range(NCORES)), trace=_trace)

    out = np.zeros((B, L, H), dtype=np.float32)
    for c in range(NCORES):
        out[c // HPC] += res.results[c]["out"]
    # v/out-proj biases commute through softmax-normalized attention exactly
    out += bv @ Wo.T + bo
    if _trace:
        kernel._last_exec_time_ns = res.exec_time_ns
        kernel._last_profile_json = res.profile_json
    return out
